# revision 6
# baseline (speedup 1.0000x reference)
"""DFlash Qwen3 cross-attention on 8 TRN2 NeuronCores.

Sharding: tensor-parallel over heads. Core c owns KV head c (KVH=8) and the
4 query heads 4c..4c+3 of its GQA group. Each core computes its heads'
QKV projections, per-head RMSNorm + RoPE, causal attention, then the
normalized per-head attention outputs (laid out transposed, [4*D, QL]) are
AllGathered so every core holds attn^T [H*D, QL]; each core then computes a
512-column slice of o_proj and the host concatenates the 8 slices.

All matmuls run in bf16 (fp32 PSUM accumulation); softmax in fp32.
Host-side prep: transpose ck=concat(context,query) to [HID, KV] bf16,
slice per-core weights, precompute RoPE cos/sin and causal mask tiles.
"""

from contextlib import ExitStack

import numpy as np
from ml_dtypes import bfloat16

import concourse.bass as bass
import concourse.bass_isa as bass_isa
import concourse.mybir as mybir
import concourse.tile as tile
from concourse import bacc
from concourse.bass_utils import run_bass_kernel_spmd
from concourse.masks import make_identity

H = 32
KVH = 8
D = 128
HID = 4096
CTX = 4096
QL = 2048
KV = CTX + QL  # 6144
NCORES = 8
HPC = H // NCORES  # 4 q heads per core
THETA = 1000000.0
EPS = 1e-6
SCALE = float(D) ** -0.5

NHD = HID // 128  # 32 contraction chunks
NKV = KV // 128  # 48 kv chunks
NQC = QL // 128  # 16 q row chunks
NQJ = QL // 512  # 4 q column tiles for attention
MASKVAL = -1e6

F32 = mybir.dt.float32
BF16 = mybir.dt.bfloat16

_STATE = {}


def _build():
    nc = bacc.Bacc()

    ckT = nc.declare_dram_parameter("ckT", [HID, KV], BF16, isOutput=False)
    wq = nc.declare_dram_parameter("wq", [HID, HPC * D], BF16, isOutput=False)
    wkv = nc.declare_dram_parameter("wkv", [HID, 2 * D], BF16, isOutput=False)
    wo = nc.declare_dram_parameter("wo", [HID, HPC * D], BF16, isOutput=False)
    cs = nc.declare_dram_parameter("cs", [KV, D], F32, isOutput=False)
    nw = nc.declare_dram_parameter("nw", [128, 2 * D], F32, isOutput=False)
    msk = nc.declare_dram_parameter("msk", [128, 4 * 512], F32, isOutput=False)
    out_ext = nc.declare_dram_parameter("out", [QL, HPC * D], F32, isOutput=True)

    ag_in = nc.dram_tensor("ag_in", [HPC * D, QL], BF16)
    ag_out = nc.dram_tensor("ag_out", [H * D, QL], BF16, addr_space="Shared")

    with tile.TileContext(nc) as tc, ExitStack() as ctx:
        singles = ctx.enter_context(tc.tile_pool(name="singles", bufs=1))
        # streamed ckT slices for the projections
        ckq_pool = ctx.enter_context(tc.tile_pool(name="ckq", bufs=4))
        wqs_pool = ctx.enter_context(tc.tile_pool(name="wqs", bufs=3))
        cs_pool = ctx.enter_context(tc.tile_pool(name="csp", bufs=3))
        # fp32 evacuation + norm/rope working tiles
        evac_pool = ctx.enter_context(tc.tile_pool(name="evac", bufs=3))
        tmp_pool = ctx.enter_context(tc.tile_pool(name="tmp", bufs=4))
        # attention-side pools
        p_pool = ctx.enter_context(tc.tile_pool(name="pt", bufs=4))
        sacc_pool = ctx.enter_context(tc.tile_pool(name="sacc", bufs=4))
        stg_pool = ctx.enter_context(tc.tile_pool(name="stg", bufs=4))
        oproj_pool = ctx.enter_context(tc.tile_pool(name="oproj", bufs=3))
        # PSUM: 4 accumulator banks + 4 shared banks (S^T chunks / transposes)
        acc_psum = ctx.enter_context(tc.tile_pool(name="accp", bufs=4, space="PSUM"))
        st_psum = ctx.enter_context(tc.tile_pool(name="stp", bufs=4, space="PSUM"))

        # ---- resident tensors ----
        wkv_sb = singles.tile([128, NHD, 2 * D], BF16)
        nc.gpsimd.dma_start(out=wkv_sb[:], in_=wkv[:, :].rearrange("(k p) n -> p k n", p=128))
        wo_sb = singles.tile([128, NHD, HPC * D], BF16)
        nc.gpsimd.dma_start(out=wo_sb[:], in_=wo[:, :].rearrange("(k p) n -> p k n", p=128))
        nw_sb = singles.tile([128, 2 * D], F32)
        nc.gpsimd.dma_start(out=nw_sb[:], in_=nw[:, :])
        msk_sb = singles.tile([128, 4 * 512], F32)
        nc.gpsimd.dma_start(out=msk_sb[:], in_=msk[:, :])

        ident = singles.tile([128, 128], F32)
        make_identity(nc, ident)
        epst = singles.tile([128, 1], F32)
        nc.vector.memset(epst, EPS)
        zbias = singles.tile([128, 1], F32)
        nc.vector.memset(zbias, 0.0)

        # outputs of the projection phases (bufs=1: written once, read later)
        qT_sb = singles.tile([128, HPC, QL], BF16)  # Q^T per head: [d, h, q]
        kT_sb = singles.tile([128, KV], BF16)  # K^T: [d, kv]
        v_sb = singles.tile([128, NKV, D], BF16)  # V: [kv%128, r, d]

        def rmsnorm_rope(xh, nw_col, cst, ro):
            """xh: [128, 128] f32 (rows = positions), normalized+roped -> ro."""
            sq = tmp_pool.tile([128, D], F32, tag="sq")
            nc.vector.tensor_mul(sq, xh, xh)
            ssum = tmp_pool.tile([128, 1], F32, tag="ssum")
            nc.vector.tensor_reduce(ssum, sq, axis=mybir.AxisListType.X, op=mybir.AluOpType.add)
            # ssum := sqrt(mean + eps); then reciprocal -> 1/rms
            nc.scalar.activation(out=ssum, in_=ssum, func=mybir.ActivationFunctionType.Sqrt,
                                 bias=epst, scale=1.0 / D)
            nc.vector.reciprocal(ssum, ssum)
            nc.vector.tensor_scalar_mul(out=xh, in0=xh, scalar1=ssum)
            nc.vector.tensor_mul(xh, xh, nw_sb[:, nw_col * D:(nw_col + 1) * D])
            c1 = cst[:, 0:64]
            s1 = cst[:, 64:128]
            t1 = tmp_pool.tile([128, 64], F32, tag="t1")
            nc.vector.tensor_mul(ro[:, 0:64], xh[:, 0:64], c1)
            nc.vector.tensor_mul(t1, xh[:, 64:128], s1)
            nc.vector.tensor_sub(ro[:, 0:64], ro[:, 0:64], t1)
            t2 = tmp_pool.tile([128, 64], F32, tag="t1")
            nc.vector.tensor_mul(ro[:, 64:128], xh[:, 64:128], c1)
            nc.vector.tensor_mul(t2, xh[:, 0:64], s1)
            nc.vector.tensor_add(ro[:, 64:128], ro[:, 64:128], t2)

        # ---- Q projection (+norm+rope+transpose) ----
        for qg in range(4):  # groups of 4 q row-chunks
            pq = [acc_psum.tile([128, HPC * D], F32, tag="acc", name=f"pq{qg}_{i}") for i in range(4)]
            for k in range(NHD):
                cqt = ckq_pool.tile([128, 512], BF16, tag="ckq")
                nc.gpsimd.dma_start(out=cqt, in_=ckT[k * 128:(k + 1) * 128,
                                                   CTX + qg * 512: CTX + (qg + 1) * 512])
                wqt = wqs_pool.tile([128, HPC * D], BF16, tag="wqs")
                nc.gpsimd.dma_start(out=wqt, in_=wq[k * 128:(k + 1) * 128, :])
                for q4 in range(4):
                    nc.tensor.matmul(pq[q4], lhsT=cqt[:, q4 * 128:(q4 + 1) * 128],
                                     rhs=wqt, start=(k == 0), stop=(k == NHD - 1))
            for q4 in range(4):
                qc = qg * 4 + q4
                qe = evac_pool.tile([128, HPC * D], F32, tag="evac")
                nc.scalar.copy(out=qe, in_=pq[q4])
                cst = cs_pool.tile([128, D], F32, tag="csp")
                nc.gpsimd.dma_start(out=cst, in_=cs[(CTX // 128 + qc) * 128:(CTX // 128 + qc + 1) * 128, :])
                for h in range(HPC):
                    ro = tmp_pool.tile([128, D], F32, tag="ro")
                    rmsnorm_rope(qe[:, h * D:(h + 1) * D], 0, cst, ro)
                    tp = st_psum.tile([128, 128], F32, tag="st")
                    nc.tensor.transpose(tp, ro, ident)
                    nc.scalar.copy(out=qT_sb[:, h, qc * 128:(qc + 1) * 128], in_=tp)

        # ---- K/V projection (+norm+rope; K transposed, V natural) ----
        for rg in range(NKV // 4):  # groups of 4 kv chunks
            pk = [acc_psum.tile([128, 2 * D], F32, tag="acc", name=f"pk{rg}_{i}") for i in range(4)]
            for k in range(NHD):
                ckt = ckq_pool.tile([128, 512], BF16, tag="ckq")
                nc.gpsimd.dma_start(out=ckt, in_=ckT[k * 128:(k + 1) * 128,
                                                   rg * 512:(rg + 1) * 512])
                for r4 in range(4):
                    nc.tensor.matmul(pk[r4], lhsT=ckt[:, r4 * 128:(r4 + 1) * 128],
                                     rhs=wkv_sb[:, k, :], start=(k == 0), stop=(k == NHD - 1))
            for r4 in range(4):
                r = rg * 4 + r4
                ke = evac_pool.tile([128, 2 * D], F32, tag="evac")
                nc.scalar.copy(out=ke, in_=pk[r4])
                cst = cs_pool.tile([128, D], F32, tag="csp")
                nc.gpsimd.dma_start(out=cst, in_=cs[r * 128:(r + 1) * 128, :])
                ro = tmp_pool.tile([128, D], F32, tag="ro")
                rmsnorm_rope(ke[:, 0:D], 1, cst, ro)
                tp = st_psum.tile([128, 128], F32, tag="st")
                nc.tensor.transpose(tp, ro, ident)
                nc.scalar.copy(out=kT_sb[:, r * 128:(r + 1) * 128], in_=tp)
                nc.vector.tensor_copy(out=v_sb[:, r, :], in_=ke[:, D:2 * D])

        # ---- attention, per local head ----
        # S^T orientation: [kv partitions, q free]; exp output IS P^T; PV with
        # V stationary gives out^T [d, q] directly.  q position of col q is
        # CTX+j*512+q; kv chunk r fully visible iff r<=31+4j, partial for
        # i=r-32-4j in 0..3, masked out beyond.
        for h in range(HPC):
            o_acc = [acc_psum.tile([128, 512], F32, tag="acc", name=f"oacc{h}_{i}") for i in range(NQJ)]
            saccs = [sacc_pool.tile([128, 512], F32, tag="sacc", name=f"sacc{h}_{i}") for i in range(NQJ)]
            for r in range(NKV):
                js = [j for j in range(NQJ) if r <= 35 + 4 * j]
                for j in js:
                    st = st_psum.tile([128, 512], F32, tag="st")
                    nc.tensor.matmul(st, lhsT=kT_sb[:, r * 128:(r + 1) * 128],
                                     rhs=qT_sb[:, h, j * 512:(j + 1) * 512],
                                     start=True, stop=True)
                    i = r - 32 - 4 * j
                    if i >= 0:
                        nc.vector.tensor_add(st, st, msk_sb[:, i * 512:(i + 1) * 512])
                    pt = p_pool.tile([128, 512], BF16, tag="pt")
                    nc.scalar.activation(out=pt, in_=st,
                                         func=mybir.ActivationFunctionType.Exp,
                                         bias=zbias, scale=SCALE)
                    if r == 0:
                        nc.vector.tensor_copy(out=saccs[j], in_=pt)
                    else:
                        nc.vector.tensor_add(saccs[j], saccs[j], pt)
                    nc.tensor.matmul(o_acc[j], lhsT=v_sb[:, r, :], rhs=pt,
                                     start=(r == 0), stop=(r == 35 + 4 * j or r == NKV - 1))
            for j in range(NQJ):
                pr = sacc_pool.tile([128, 512], F32, tag="pr")
                nc.gpsimd.partition_all_reduce(pr, saccs[j], channels=128,
                                               reduce_op=bass_isa.ReduceOp.add)
                nc.vector.reciprocal(pr, pr)
                stg = stg_pool.tile([128, 512], BF16, tag="stg")
                nc.vector.tensor_mul(stg, o_acc[j], pr)
                nc.gpsimd.dma_start(out=ag_in[h * 128:(h + 1) * 128, j * 512:(j + 1) * 512],
                                  in_=stg)

        # ---- AllGather attn^T across the 8 cores ----
        nc.gpsimd.collective_compute(
            "AllGather",
            mybir.AluOpType.bypass,
            ins=[ag_in[:]],
            outs=[ag_out[:]],
            replica_groups=[list(range(NCORES))],
        )

        # ---- o_proj: out[:, c*512:(c+1)*512] = attn @ wo_c ----
        for qc in range(NQC):
            at = oproj_pool.tile([128, NHD, 128], BF16, tag="at")
            nc.gpsimd.dma_start(
                out=at,
                in_=ag_out[:, qc * 128:(qc + 1) * 128].rearrange("(k p) q -> p k q", p=128))
            po = acc_psum.tile([128, HPC * D], F32, tag="acc")
            for k in range(NHD):
                nc.tensor.matmul(po, lhsT=at[:, k, :], rhs=wo_sb[:, k, :],
                                 start=(k == 0), stop=(k == NHD - 1))
            ot = stg_pool.tile([128, HPC * D], F32, tag="ot")
            nc.scalar.copy(out=ot, in_=po)
            nc.gpsimd.dma_start(out=out_ext[qc * 128:(qc + 1) * 128, :], in_=ot)

    nc.compile()
    return nc


def _host_prep(context, query, w_qkv, w_o, q_norm_w, k_norm_w):
    context = np.asarray(context, dtype=np.float32)
    query = np.asarray(query, dtype=np.float32)
    w_qkv = np.asarray(w_qkv, dtype=np.float32)
    w_o = np.asarray(w_o, dtype=np.float32)
    q_norm_w = np.asarray(q_norm_w, dtype=np.float32)
    k_norm_w = np.asarray(k_norm_w, dtype=np.float32)

    ck = np.concatenate([context, query], axis=0)  # [KV, HID]
    ckT = np.ascontiguousarray(ck.T).astype(bfloat16)  # [HID, KV]

    wq = w_qkv[:, :H * D]
    wk = w_qkv[:, H * D:H * D + KVH * D]
    wv = w_qkv[:, H * D + KVH * D:]

    half = D // 2
    inv_freq = (1.0 / (THETA ** (np.arange(0, half, dtype=np.float32) / half))).astype(np.float32)
    pos = np.arange(KV, dtype=np.float32)
    freqs = pos[:, None] * inv_freq[None, :]
    cs = np.concatenate([np.cos(freqs), np.sin(freqs)], axis=1).astype(np.float32)  # [KV, D]

    nw = np.concatenate([
        np.broadcast_to(q_norm_w[None, :], (128, D)),
        np.broadcast_to(k_norm_w[None, :], (128, D)),
    ], axis=1).astype(np.float32)  # [128, 2D]

    p = np.arange(128)[:, None]
    q = np.arange(512)[None, :]
    msk = np.concatenate(
        [np.where(128 * i + p <= q, 0.0, MASKVAL) for i in range(4)],
        axis=1).astype(np.float32)  # [128, 2048]

    in_maps = []
    for c in range(NCORES):
        in_maps.append({
            "ckT": ckT,
            "wq": np.ascontiguousarray(wq[:, c * HPC * D:(c + 1) * HPC * D]).astype(bfloat16),
            "wkv": np.ascontiguousarray(
                np.concatenate([wk[:, c * D:(c + 1) * D], wv[:, c * D:(c + 1) * D]], axis=1)
            ).astype(bfloat16),
            "wo": np.ascontiguousarray(w_o[:, c * HPC * D:(c + 1) * HPC * D]).astype(bfloat16),
            "cs": cs,
            "nw": nw,
            "msk": msk,
        })
    return in_maps


def kernel(context, query, w_qkv, w_o, q_norm_w, k_norm_w, **kw):
    if "nc" not in _STATE:
        _STATE["nc"] = _build()
    nc = _STATE["nc"]
    in_maps = _host_prep(context, query, w_qkv, w_o, q_norm_w, k_norm_w)
    res = run_bass_kernel_spmd(nc, in_maps, list(range(NCORES)), **kw)
    out = np.concatenate([np.asarray(res.results[c]["out"]) for c in range(NCORES)], axis=1)
    if kw:
        return out.astype(np.float32), res
    return out.astype(np.float32)
